# revision 4
# baseline (speedup 1.0000x reference)
"""AGNN (2-layer propagation) Trainium2 Bass kernel, 8-core SPMD.

Sharding: destination-node ranges across 8 cores (12500 nodes/core, padded to
12544 = 98 tiles of 128), per the dst-range graph-parallel strategy. Per core:
  - h0 = relu(x_local @ W1 + b1) via PE (W1 stationary, x^T moving).
  - node records (hn||h, bf16) packed 4 nodes per 256B window -> DRAM table,
    AllGather across cores (window index fits int16 for dma_gather).
  - per-edge phase in ELL layout (dst-node-major, column-major slot stream):
    gpsimd dma_gather fetches source windows; DVE/ACT compute the
    segment softmax (shift-free: logits = beta*cos are bounded) and the
    weighted sums; per-dst reductions via tensor_reduce.
  - classifier + log_softmax on local nodes; host un-permutes and concats.
"""
import sys
import types

sys.path.insert(0, "/opt/trn_rl_repo")

import numpy as np

try:  # optional NTFF profiling hook (enabled by test.py via TRACE)
    import antenv
    if "antenv.axon_hooks" not in sys.modules:
        _hook = [None]
        _m = types.ModuleType("antenv.axon_hooks")
        _m.set_axon_ntff_profile_hook = lambda h: _hook.__setitem__(0, h)
        _m.get_axon_ntff_profile_hook = lambda: _hook[0]
        sys.modules["antenv.axon_hooks"] = _m
        antenv.axon_hooks = _m
        try:
            from trn_agent_boot.trn_boot import _ntff_profile_via_ctypes
            _m.set_axon_ntff_profile_hook(
                _ntff_profile_via_ctypes("/opt/axon/libaxon_pjrt.so"))
        except Exception:
            pass
except Exception:
    pass

import concourse.bass as bass  # noqa: F401
import concourse.mybir as mybir
import concourse.tile as tile
from concourse import bacc
from concourse import library_config
from concourse.bass_utils import run_bass_kernel_spmd
from concourse.masks import make_identity

F32 = mybir.dt.float32
BF16 = mybir.dt.bfloat16
I16 = mybir.dt.int16
AF = mybir.ActivationFunctionType
OP = mybir.AluOpType
AX = mybir.AxisListType

NC_CORES = 8
N = 100000
F_IN = 1433
H = 16
C = 7
EPS = 1e-12

L = N // NC_CORES            # 12500 local nodes per core
LP = 12544                   # padded (98 tiles of 128)
NT = LP // 128               # 98 tiles
KP = 1536                    # padded contraction dim (12 x 128)
NW_CORE = LP // 4            # 3136 windows per core
NW = NC_CORES * NW_CORE      # 25088 real windows
DUMMY_W = NW                 # zero window
NTAB = NW + 4
NIDX_CALL = 1024             # dma_gather safe per-call index count
COLS_CALL = NIDX_CALL // 128  # 8 ELL columns per call
COL_W = NIDX_CALL // 16       # 64 wrapped idx columns per call
GROUP_CAP = 88               # max summed K per gather group (cols)

TRACE = [False]
LAST_EXEC_NS = [None]


def _bc(ap, shape):
    try:
        return ap.broadcast_to(shape)
    except Exception:
        return ap.to_broadcast(shape)


def _plan(deg_sorted):
    """Shared (across cores) tile K profile and gather group/call layout."""
    Kprof = np.maximum(deg_sorted[:, ::128].max(axis=0), 1).astype(np.int64)
    groups = []          # (tile_list, ncalls, cap, col_base)
    colbase = np.zeros(NT, dtype=np.int64)
    callbase = []
    cols_total = 0
    calls_total = 0
    t = 0
    while t < NT:
        ts = [t]
        sk = int(Kprof[t])
        t += 1
        while t < NT and sk + int(Kprof[t]) <= GROUP_CAP:
            sk += int(Kprof[t])
            ts.append(t)
            t += 1
        ncalls = (sk + COLS_CALL - 1) // COLS_CALL
        cap = ncalls * COLS_CALL
        off = 0
        for tt in ts:
            colbase[tt] = cols_total + off
            off += int(Kprof[tt])
        callbase.append(calls_total)
        groups.append((ts, ncalls, cap, cols_total))
        cols_total += cap
        calls_total += ncalls
    return Kprof, groups, colbase, callbase, cols_total, calls_total


def _host_prep(x, edge_index, W1, b1, beta, W2, b2):
    deg = np.bincount(edge_index[1], minlength=N) + 1  # +self loop

    perms = []
    deg_sorted = np.zeros((NC_CORES, LP), dtype=np.int64)
    for c in range(NC_CORES):
        dl = deg[c * L:(c + 1) * L]
        order = np.argsort(-dl, kind="stable")
        perms.append(order)
        deg_sorted[c, :L] = dl[order]

    Kprof, groups, colbase, callbase, cols_total, calls_total = _plan(deg_sorted)
    KMAX = int(Kprof.max())

    src_all = np.concatenate([edge_index[0].astype(np.int64),
                              np.arange(N, dtype=np.int64)])
    dst_all = np.concatenate([edge_index[1].astype(np.int64),
                              np.arange(N, dtype=np.int64)])

    rank_of = np.empty(N, dtype=np.int64)
    for c in range(NC_CORES):
        rank_of[c * L + perms[c]] = np.arange(L)
    src_rank = rank_of[src_all]
    src_gp = (src_all // L) * LP + (src_rank % 128) * NT + (src_rank // 128)
    dstc = dst_all // L
    dst_rank = rank_of[dst_all]

    idx_streams = np.empty((NC_CORES, 128, calls_total * COL_W), dtype=np.int16)
    subv = np.zeros((NC_CORES, 128, cols_total), dtype=np.float32)

    for c in range(NC_CORES):
        sel = dstc == c
        dr = dst_rank[sel]
        gp = src_gp[sel]
        o = np.argsort(dr, kind="stable")
        dr = dr[o]; gp = gp[o]
        starts = np.searchsorted(dr, np.arange(LP))
        pos = np.arange(len(dr)) - starts[dr]
        tt = dr // 128
        pp = dr % 128
        col = colbase[tt] + pos
        iw = np.full((128, cols_total), DUMMY_W, dtype=np.int64)
        iw[pp, col] = gp >> 2
        subv[c][pp, col] = gp & 3
        for gi, (ts, ncalls, cap, colb) in enumerate(groups):
            blk = iw[:, colb:colb + cap]                   # [128, cap]
            lin = blk.T.reshape(ncalls, NIDX_CALL)         # col-major per call
            wr = lin.reshape(ncalls, COL_W, 16).transpose(0, 2, 1)
            flat = wr.transpose(1, 0, 2).reshape(16, ncalls * COL_W)
            cb = callbase[gi]
            for rep in range(8):
                idx_streams[c, rep * 16:(rep + 1) * 16,
                            cb * COL_W:(cb + ncalls) * COL_W] = flat

    deg_pt = np.zeros((NC_CORES, 128, NT), dtype=np.float32)
    for c in range(NC_CORES):
        deg_pt[c] = deg_sorted[c].reshape(NT, 128).T  # [p, t]

    iota = np.tile(np.arange(KMAX, dtype=np.float32), (128, 1))

    in_maps = []
    W1p = np.zeros((KP, H), dtype=np.float32)
    W1p[:F_IN] = W1
    for c in range(NC_CORES):
        xt = np.zeros((KP, LP), dtype=np.float32)
        xt[:F_IN, :L] = x[c * L + perms[c]].T
        in_maps.append({
            "xt": xt,
            "w1": W1p,
            "b1": b1.reshape(H, 1).astype(np.float32),
            "w2": W2.astype(np.float32),
            "b2r": np.tile(b2.reshape(1, C), (128, 1)).astype(np.float32),
            "betar": np.full((128, 1), float(beta[0]), dtype=np.float32),
            "idxs": idx_streams[c],
            "subv": subv[c],
            "degs": deg_pt[c],
            "iota": iota,
        })
    meta = dict(groups=groups, colbase=colbase, callbase=callbase,
                cols_total=cols_total, calls_total=calls_total,
                Kprof=Kprof, KMAX=KMAX, perms=perms)
    return in_maps, meta


def _build_program(meta):
    groups = meta["groups"]
    colbase = meta["colbase"]
    callbase = meta["callbase"]
    cols_total = meta["cols_total"]
    calls_total = meta["calls_total"]
    Kprof = meta["Kprof"]
    KMAX = meta["KMAX"]
    CAPMAX = int(max(g[2] for g in groups))
    NCALLS_MAX = int(max(g[1] for g in groups))

    nc = bacc.Bacc("TRN2", target_bir_lowering=False, debug=False,
                   num_devices=NC_CORES)

    xt_d = nc.dram_tensor("xt", [KP, LP], F32, kind="ExternalInput")
    w1_d = nc.dram_tensor("w1", [KP, H], F32, kind="ExternalInput")
    b1_d = nc.dram_tensor("b1", [H, 1], F32, kind="ExternalInput")
    w2_d = nc.dram_tensor("w2", [H, C], F32, kind="ExternalInput")
    b2r_d = nc.dram_tensor("b2r", [128, C], F32, kind="ExternalInput")
    betar_d = nc.dram_tensor("betar", [128, 1], F32, kind="ExternalInput")
    idxs_d = nc.dram_tensor("idxs", [128, calls_total * COL_W], I16,
                            kind="ExternalInput")
    subv_d = nc.dram_tensor("subv", [128, cols_total], F32,
                            kind="ExternalInput")
    degs_d = nc.dram_tensor("degs", [128, NT], F32, kind="ExternalInput")
    iota_d = nc.dram_tensor("iota", [128, KMAX], F32, kind="ExternalInput")
    out_d = nc.dram_tensor("out", [LP, C], F32, kind="ExternalOutput")

    rec_d = [nc.dram_tensor(f"rec{i}", [128, NT, 32], BF16) for i in range(2)]
    tab_d = [nc.dram_tensor(f"tab{i}", [NTAB, 128], BF16, addr_space="Shared")
             for i in range(2)]

    with tile.TileContext(nc) as tc:
        with tc.tile_pool(name="const", bufs=1) as cst, \
             tc.tile_pool(name="state", bufs=1) as st, \
             tc.tile_pool(name="work", bufs=2) as wk, \
             tc.tile_pool(name="gath", bufs=2) as gp:

            nc.gpsimd.load_library(library_config.mlp)

            w1sb = cst.tile([128, 12, H], F32)
            for kt in range(12):
                nc.sync.dma_start(out=w1sb[:, kt, :],
                                  in_=w1_d[kt * 128:(kt + 1) * 128, :])
            b1sb = cst.tile([H, 1], F32)
            nc.sync.dma_start(out=b1sb[:], in_=b1_d[:])
            w2sb = cst.tile([H, C], F32)
            nc.sync.dma_start(out=w2sb[:], in_=w2_d[:])
            b2rsb = cst.tile([128, C], F32)
            nc.sync.dma_start(out=b2rsb[:], in_=b2r_d[:])
            betasb = cst.tile([128, 1], F32)
            nc.sync.dma_start(out=betasb[:], in_=betar_d[:])
            subsb = cst.tile([128, cols_total], F32)
            nc.sync.dma_start(out=subsb[:], in_=subv_d[:])
            degsb = cst.tile([128, NT], F32)
            nc.sync.dma_start(out=degsb[:], in_=degs_d[:])
            iotasb = cst.tile([128, KMAX], F32)
            nc.sync.dma_start(out=iotasb[:], in_=iota_d[:])
            ident128 = cst.tile([128, 128], F32)
            make_identity(nc, ident128[:])
            zer = cst.tile([1, 128], BF16)
            nc.vector.memset(zer[:], 0)
            for i in range(2):
                nc.sync.dma_start(out=tab_d[i][NW:NW + 1, :], in_=zer[:])

            # ------------- phase A: h0 = relu(x W1 + b1), node-major -------
            h0nm = st.tile([128, NT, H], F32)
            with tc.tile_pool(name="psA", bufs=1, space="PSUM") as psA, \
                 tc.tile_pool(name="psTa", bufs=4, space="PSUM") as psTa, \
                 tc.tile_pool(name="wkA", bufs=2) as wkA:
                CH = 2048
                for coff in range(0, LP, CH):
                    csz = min(CH, LP - coff)
                    ps = psA.tile([H, CH], F32, tag="psa")
                    for kt in range(12):
                        xtile = wkA.tile([128, CH], F32, tag="xt")
                        nc.sync.dma_start(
                            out=xtile[:, :csz],
                            in_=xt_d[kt * 128:(kt + 1) * 128, coff:coff + csz])
                        for m in range(0, csz, 512):
                            mw = min(512, csz - m)
                            nc.tensor.matmul(ps[:, m:m + mw],
                                             lhsT=w1sb[:, kt, :],
                                             rhs=xtile[:, m:m + mw],
                                             start=(kt == 0), stop=(kt == 11))
                    hfm = wkA.tile([H, CH], F32, tag="hfm")
                    nc.scalar.activation(hfm[:, :csz], ps[:, :csz], AF.Relu,
                                         bias=b1sb[:])
                    for i in range(csz // 128):
                        tg = (coff + i * 128) // 128
                        pt = psTa.tile([128, H], F32, tag="pst")
                        nc.tensor.transpose(
                            out=pt[:], in_=hfm[:, i * 128:(i + 1) * 128],
                            identity=ident128[:H, :H])
                        nc.vector.tensor_copy(out=h0nm[:, tg, :], in_=pt[:])

            hnbf = st.tile([128, NT, H], BF16)
            h1nm = st.tile([128, NT, H], F32)
            sgrp = st.tile([128, NT], F32)

            def normalize_and_share(hsrc, phase):
                hh = wk.tile([128, NT * H], F32, tag="hh")
                nc.scalar.activation(
                    hh[:], hsrc[:].rearrange("p t h -> p (t h)"), AF.Square)
                ss = wk.tile([128, NT], F32, tag="ss")
                nc.vector.tensor_reduce(
                    ss[:], hh[:].rearrange("p (t h) -> p t h", h=H),
                    axis=AX.X, op=OP.add)
                nc.vector.tensor_scalar_add(ss[:], ss[:], EPS)
                sq = wk.tile([128, NT], F32, tag="ss2")
                nc.scalar.activation(sq[:], ss[:], AF.Sqrt)
                rr = wk.tile([128, NT], F32, tag="rr")
                nc.vector.reciprocal(rr[:], sq[:])
                hnf = wk.tile([128, NT, H], F32, tag="hnf")
                nc.vector.tensor_tensor(
                    out=hnf[:], in0=hsrc[:],
                    in1=_bc(rr[:].unsqueeze(2), [128, NT, H]),
                    op=OP.mult)
                nc.vector.tensor_copy(out=hnbf[:], in_=hnf[:])
                rec = wk.tile([128, NT, 32], BF16, tag="rec")
                nc.vector.tensor_copy(out=rec[:, :, 0:H], in_=hnf[:])
                nc.vector.tensor_copy(out=rec[:, :, H:2 * H], in_=hsrc[:])
                nc.sync.dma_start(out=rec_d[phase][:], in_=rec[:])
                nc.gpsimd.collective_compute(
                    "AllGather", OP.bypass,
                    replica_groups=[list(range(NC_CORES))],
                    ins=[rec_d[phase][:]],
                    outs=[tab_d[phase][0:NW, :]],
                )

            def prop(hio, phase, use_beta):
                for gi, (ts, ncalls, cap, colb) in enumerate(groups):
                    idxsb = wk.tile([128, NCALLS_MAX * COL_W], I16, tag="idx")
                    cb = callbase[gi]
                    nc.sync.dma_start(
                        out=idxsb[:, :ncalls * COL_W],
                        in_=idxs_d[:, cb * COL_W:(cb + ncalls) * COL_W])
                    G4 = gp.tile([128, CAPMAX, 128], BF16, tag="g4")
                    for cc in range(ncalls):
                        nc.gpsimd.dma_gather(
                            out_ap=G4[:, cc * COLS_CALL:(cc + 1) * COLS_CALL, :],
                            in_ap=tab_d[phase][:],
                            idxs_ap=idxsb[:, cc * COL_W:(cc + 1) * COL_W],
                            num_idxs=NIDX_CALL,
                            num_idxs_reg=NIDX_CALL,
                            elem_size=128,
                        )
                    off = 0
                    for t in ts:
                        K = int(Kprof[t])
                        Gt = G4[:, off:off + K, :]
                        Gs = wk.tile([128, KMAX, 32], BF16, tag="gs")
                        nc.vector.tensor_copy(out=Gs[:, :K, :],
                                              in_=Gt[:, :, 0:32])
                        for j in (1, 2, 3):
                            mj = wk.tile([128, KMAX], mybir.dt.uint8, tag="mj")
                            nc.vector.tensor_scalar(
                                out=mj[:, :K],
                                in0=subsb[:, colbase[t]:colbase[t] + K],
                                scalar1=float(j), scalar2=None,
                                op0=OP.is_equal)
                            nc.vector.copy_predicated(
                                out=Gs[:, :K, :],
                                mask=_bc(mj[:, :K].unsqueeze(2),
                                         [128, K, 32]),
                                data=Gt[:, :, 32 * j:32 * j + 32])
                        prod = wk.tile([128, KMAX, H], BF16, tag="prod")
                        nc.vector.tensor_tensor(
                            out=prod[:, :K, :], in0=Gs[:, :K, 0:H],
                            in1=_bc(hnbf[:, t:t + 1, :], [128, K, H]),
                            op=OP.mult)
                        cosr = wk.tile([128, KMAX], F32, tag="cosr")
                        nc.vector.tensor_reduce(cosr[:, :K], prod[:, :K, :],
                                                axis=AX.X, op=OP.add)
                        if use_beta:
                            nc.vector.tensor_scalar_mul(cosr[:, :K],
                                                        cosr[:, :K],
                                                        betasb[:])
                        ee = wk.tile([128, KMAX], F32, tag="ee")
                        nc.scalar.activation(ee[:, :K], cosr[:, :K], AF.Exp)
                        msk = wk.tile([128, KMAX], F32, tag="msk")
                        nc.vector.tensor_tensor(
                            out=msk[:, :K],
                            in0=_bc(degsb[:, t:t + 1], [128, K]),
                            in1=iotasb[:, :K], op=OP.is_gt)
                        em = wk.tile([128, KMAX], F32, tag="em")
                        nc.vector.tensor_tensor(out=em[:, :K], in0=ee[:, :K],
                                                in1=msk[:, :K], op=OP.mult)
                        nc.vector.tensor_reduce(sgrp[:, t:t + 1], em[:, :K],
                                                axis=AX.X, op=OP.add)
                        wei = wk.tile([128, KMAX, H], BF16, tag="wei")
                        nc.vector.tensor_tensor(
                            out=wei[:, :K, :], in0=Gs[:, :K, H:2 * H],
                            in1=_bc(em[:, :K].unsqueeze(2),
                                    [128, K, H]),
                            op=OP.mult)
                        nc.vector.tensor_reduce(
                            h1nm[:, t, :],
                            wei[:, :K, :].rearrange("p k h -> p h k"),
                            axis=AX.X, op=OP.add)
                        off += K
                nc.vector.tensor_scalar_max(sgrp[:], sgrp[:], 1e-30)
                rs = wk.tile([128, NT], F32, tag="rs")
                nc.vector.reciprocal(rs[:], sgrp[:])
                nc.vector.tensor_tensor(
                    out=hio[:], in0=h1nm[:],
                    in1=_bc(rs[:].unsqueeze(2), [128, NT, H]),
                    op=OP.mult)

            normalize_and_share(h0nm, 0)
            prop(h0nm, 0, use_beta=False)
            normalize_and_share(h0nm, 1)
            prop(h0nm, 1, use_beta=True)

            # ------------- classifier + log_softmax -----------------------
            logits = st.tile([128, NT, C], F32)
            with tc.tile_pool(name="psC", bufs=2, space="PSUM") as psC:
                for t in range(NT):
                    ptx = psC.tile([H, 128], F32, tag="ptx")
                    nc.tensor.transpose(out=ptx[:], in_=h0nm[:, t, :],
                                        identity=ident128[:])
                    h2T = wk.tile([H, 128], F32, tag="h2t")
                    nc.vector.tensor_copy(out=h2T[:], in_=ptx[:])
                    pl = psC.tile([128, C], F32, tag="pl")
                    nc.tensor.matmul(pl[:], lhsT=h2T[:], rhs=w2sb[:],
                                     start=True, stop=True)
                    nc.vector.tensor_tensor(out=logits[:, t, :], in0=pl[:],
                                            in1=b2rsb[:], op=OP.add)
            m7 = wk.tile([128, NT], F32, tag="m7")
            nc.vector.tensor_reduce(m7[:], logits[:], axis=AX.X, op=OP.max)
            zm = wk.tile([128, NT, C], F32, tag="zm")
            nc.vector.tensor_tensor(
                out=zm[:], in0=logits[:],
                in1=_bc(m7[:].unsqueeze(2), [128, NT, C]),
                op=OP.subtract)
            ez = wk.tile([128, NT, C], F32, tag="ez")
            nc.scalar.activation(ez[:].rearrange("p t c -> p (t c)"),
                                 zm[:].rearrange("p t c -> p (t c)"), AF.Exp)
            s7 = wk.tile([128, NT], F32, tag="s7")
            nc.vector.tensor_reduce(s7[:], ez[:], axis=AX.X, op=OP.add)
            l7 = wk.tile([128, NT], F32, tag="l7")
            nc.scalar.activation(l7[:], s7[:], AF.Ln)
            outsb = wk.tile([128, NT, C], F32, tag="outsb")
            nc.vector.tensor_tensor(
                out=outsb[:], in0=zm[:],
                in1=_bc(l7[:].unsqueeze(2), [128, NT, C]),
                op=OP.subtract)
            nc.sync.dma_start(
                out=out_d[:].rearrange("(p t) c -> p t c", p=128),
                in_=outsb[:])

    nc.compile()
    return nc


_CACHE = {}


def kernel(x, edge_index, W1, b1, beta, W2, b2):
    x = np.asarray(x, dtype=np.float32)
    edge_index = np.asarray(edge_index)
    in_maps, meta = _host_prep(x, edge_index, np.asarray(W1), np.asarray(b1),
                               np.asarray(beta), np.asarray(W2),
                               np.asarray(b2))
    if "prog" not in _CACHE:
        _CACHE["prog"] = _build_program(meta)
    nc = _CACHE["prog"]
    res = run_bass_kernel_spmd(nc, in_maps, list(range(NC_CORES)),
                               trace=TRACE[0])
    LAST_EXEC_NS[0] = res.exec_time_ns
    out = np.empty((N, C), dtype=np.float32)
    r = np.arange(L)
    for c in range(NC_CORES):
        oc = res.results[c]["out"].reshape(128, NT, C)
        out[c * L + meta["perms"][c]] = oc[r % 128, r // 128]
    return out


# revision 6
# speedup vs baseline: 2.2947x; 2.2947x over previous
"""AGNN (2-layer propagation) Trainium2 Bass kernel, 8-core SPMD.

Sharding: destination-node ranges across 8 cores (12500 nodes/core, padded to
12544 = 98 tiles of 128), per the dst-range graph-parallel strategy. Per core:
  - h0 = relu(x_local @ W1 + b1) via PE (W1 stationary, x^T moving).
  - node records (hn||h, bf16) packed 4 nodes per 256B window -> DRAM table,
    AllGather across cores (window index fits int16 for dma_gather).
  - per-edge phase in ELL layout (dst-node-major, column-major slot stream):
    gpsimd dma_gather fetches source windows; DVE/ACT compute the
    segment softmax (shift-free: logits = beta*cos are bounded) and the
    weighted sums; per-dst reductions via tensor_reduce.
  - classifier + log_softmax on local nodes; host un-permutes and concats.
"""
import sys
import types

sys.path.insert(0, "/opt/trn_rl_repo")

import numpy as np

try:  # optional NTFF profiling hook (enabled by test.py via TRACE)
    import antenv
    if "antenv.axon_hooks" not in sys.modules:
        _hook = [None]
        _m = types.ModuleType("antenv.axon_hooks")
        _m.set_axon_ntff_profile_hook = lambda h: _hook.__setitem__(0, h)
        _m.get_axon_ntff_profile_hook = lambda: _hook[0]
        sys.modules["antenv.axon_hooks"] = _m
        antenv.axon_hooks = _m
        try:
            from trn_agent_boot.trn_boot import _ntff_profile_via_ctypes
            _m.set_axon_ntff_profile_hook(
                _ntff_profile_via_ctypes("/opt/axon/libaxon_pjrt.so"))
        except Exception:
            pass
except Exception:
    pass

import concourse.bass as bass  # noqa: F401
import concourse.mybir as mybir
import concourse.tile as tile
from concourse import bacc
from concourse import library_config
from concourse.bass_utils import run_bass_kernel_spmd
from concourse.masks import make_identity

F32 = mybir.dt.float32
BF16 = mybir.dt.bfloat16
I16 = mybir.dt.int16
AF = mybir.ActivationFunctionType
OP = mybir.AluOpType
AX = mybir.AxisListType

NC_CORES = 8
N = 100000
F_IN = 1433
H = 16
C = 7
EPS = 1e-12

L = N // NC_CORES            # 12500 local nodes per core
LP = 12544                   # padded (98 tiles of 128)
NT = LP // 128               # 98 tiles
KP = 1536                    # padded contraction dim (12 x 128)
NW_CORE = LP // 4            # 3136 windows per core
NW = NC_CORES * NW_CORE      # 25088 real windows
DUMMY_W = NW                 # zero window
NTAB = NW + 4
NIDX_CALL = 1024             # dma_gather safe per-call index count
COLS_CALL = NIDX_CALL // 128  # 8 ELL columns per call
COL_W = NIDX_CALL // 16       # 64 wrapped idx columns per call
GROUP_CAP = 88               # max summed K per gather group (cols)

TRACE = [False]
LAST_EXEC_NS = [None]


def _bc(ap, shape):
    try:
        return ap.broadcast_to(shape)
    except Exception:
        return ap.to_broadcast(shape)


def _plan(deg_sorted):
    """Shared (across cores) tile K profile and gather group/call layout."""
    Kprof = np.maximum(deg_sorted[:, ::128].max(axis=0), 1).astype(np.int64)
    groups = []          # (tile_list, ncalls, cap, col_base)
    colbase = np.zeros(NT, dtype=np.int64)
    callbase = []
    cols_total = 0
    calls_total = 0
    t = 0
    while t < NT:
        ts = [t]
        sk = int(Kprof[t])
        t += 1
        while t < NT and sk + int(Kprof[t]) <= GROUP_CAP:
            sk += int(Kprof[t])
            ts.append(t)
            t += 1
        ncalls = (sk + COLS_CALL - 1) // COLS_CALL
        cap = ncalls * COLS_CALL
        off = 0
        for tt in ts:
            colbase[tt] = cols_total + off
            off += int(Kprof[tt])
        callbase.append(calls_total)
        groups.append((ts, ncalls, cap, cols_total))
        cols_total += cap
        calls_total += ncalls
    return Kprof, groups, colbase, callbase, cols_total, calls_total


def _host_prep(x, edge_index, W1, b1, beta, W2, b2):
    deg = np.bincount(edge_index[1], minlength=N) + 1  # +self loop

    perms = []
    deg_sorted = np.zeros((NC_CORES, LP), dtype=np.int64)
    for c in range(NC_CORES):
        dl = deg[c * L:(c + 1) * L]
        order = np.argsort(-dl, kind="stable")
        perms.append(order)
        deg_sorted[c, :L] = dl[order]

    Kprof, groups, colbase, callbase, cols_total, calls_total = _plan(deg_sorted)
    KMAX = int(Kprof.max())

    src_all = np.concatenate([edge_index[0].astype(np.int64),
                              np.arange(N, dtype=np.int64)])
    dst_all = np.concatenate([edge_index[1].astype(np.int64),
                              np.arange(N, dtype=np.int64)])

    rank_of = np.empty(N, dtype=np.int64)
    for c in range(NC_CORES):
        rank_of[c * L + perms[c]] = np.arange(L)
    src_rank = rank_of[src_all]
    src_gp = (src_all // L) * LP + (src_rank % 128) * NT + (src_rank // 128)
    dstc = dst_all // L
    dst_rank = rank_of[dst_all]

    idx_streams = np.empty((NC_CORES, 128, calls_total * COL_W), dtype=np.int16)
    subv = np.zeros((NC_CORES, 128, cols_total), dtype=np.float32)

    for c in range(NC_CORES):
        sel = dstc == c
        dr = dst_rank[sel]
        gp = src_gp[sel]
        o = np.argsort(dr, kind="stable")
        dr = dr[o]; gp = gp[o]
        starts = np.searchsorted(dr, np.arange(LP))
        pos = np.arange(len(dr)) - starts[dr]
        tt = dr // 128
        pp = dr % 128
        col = colbase[tt] + pos
        iw = np.full((128, cols_total), DUMMY_W, dtype=np.int64)
        iw[pp, col] = gp >> 2
        subv[c][pp, col] = gp & 3
        for gi, (ts, ncalls, cap, colb) in enumerate(groups):
            blk = iw[:, colb:colb + cap]                   # [128, cap]
            lin = blk.T.reshape(ncalls, NIDX_CALL)         # col-major per call
            wr = lin.reshape(ncalls, COL_W, 16).transpose(0, 2, 1)
            flat = wr.transpose(1, 0, 2).reshape(16, ncalls * COL_W)
            cb = callbase[gi]
            for rep in range(8):
                idx_streams[c, rep * 16:(rep + 1) * 16,
                            cb * COL_W:(cb + ncalls) * COL_W] = flat

    deg_pt = np.zeros((NC_CORES, 128, NT), dtype=np.float32)
    for c in range(NC_CORES):
        deg_pt[c] = deg_sorted[c].reshape(NT, 128).T  # [p, t]

    iota = np.tile(np.arange(KMAX, dtype=np.float32), (128, 1))

    in_maps = []
    W1p = np.zeros((KP, H), dtype=np.float32)
    W1p[:F_IN] = W1
    for c in range(NC_CORES):
        xt = np.zeros((KP, LP), dtype=np.float32)
        xt[:F_IN, :L] = x[c * L + perms[c]].T
        in_maps.append({
            "xt": xt,
            "w1": W1p,
            "b1": b1.reshape(H, 1).astype(np.float32),
            "w2": W2.astype(np.float32),
            "b2r": np.tile(b2.reshape(1, C), (128, 1)).astype(np.float32),
            "betar": np.full((128, 1), float(beta[0]), dtype=np.float32),
            "idxs": idx_streams[c],
            "subv": subv[c],
            "degs": deg_pt[c],
            "iota": iota,
        })
    meta = dict(groups=groups, colbase=colbase, callbase=callbase,
                cols_total=cols_total, calls_total=calls_total,
                Kprof=Kprof, KMAX=KMAX, perms=perms)
    return in_maps, meta


def _build_program(meta):
    groups = meta["groups"]
    colbase = meta["colbase"]
    callbase = meta["callbase"]
    cols_total = meta["cols_total"]
    calls_total = meta["calls_total"]
    Kprof = meta["Kprof"]
    KMAX = meta["KMAX"]
    CAPMAX = int(max(g[2] for g in groups))
    NCALLS_MAX = int(max(g[1] for g in groups))

    nc = bacc.Bacc("TRN2", target_bir_lowering=False, debug=False,
                   num_devices=NC_CORES, num_swdge_queues=4)

    xt_d = nc.dram_tensor("xt", [KP, LP], F32, kind="ExternalInput")
    w1_d = nc.dram_tensor("w1", [KP, H], F32, kind="ExternalInput")
    b1_d = nc.dram_tensor("b1", [H, 1], F32, kind="ExternalInput")
    w2_d = nc.dram_tensor("w2", [H, C], F32, kind="ExternalInput")
    b2r_d = nc.dram_tensor("b2r", [128, C], F32, kind="ExternalInput")
    betar_d = nc.dram_tensor("betar", [128, 1], F32, kind="ExternalInput")
    idxs_d = nc.dram_tensor("idxs", [128, calls_total * COL_W], I16,
                            kind="ExternalInput")
    subv_d = nc.dram_tensor("subv", [128, cols_total], F32,
                            kind="ExternalInput")
    degs_d = nc.dram_tensor("degs", [128, NT], F32, kind="ExternalInput")
    iota_d = nc.dram_tensor("iota", [128, KMAX], F32, kind="ExternalInput")
    out_d = nc.dram_tensor("out", [LP, C], F32, kind="ExternalOutput")

    rec_d = [nc.dram_tensor(f"rec{i}", [128, NT, 32], BF16) for i in range(2)]
    tab_d = [nc.dram_tensor(f"tab{i}", [NTAB, 128], BF16, addr_space="Shared")
             for i in range(2)]

    with tile.TileContext(nc) as tc:
        with tc.tile_pool(name="const", bufs=1) as cst, \
             tc.tile_pool(name="state", bufs=1) as st, \
             tc.tile_pool(name="work", bufs=2) as wk, \
             tc.tile_pool(name="gath", bufs=2) as gp:

            nc.gpsimd.load_library(library_config.mlp)

            w1sb = cst.tile([128, 12, H], F32)
            for kt in range(12):
                nc.sync.dma_start(out=w1sb[:, kt, :],
                                  in_=w1_d[kt * 128:(kt + 1) * 128, :])
            b1sb = cst.tile([H, 1], F32)
            nc.sync.dma_start(out=b1sb[:], in_=b1_d[:])
            w2sb = cst.tile([H, C], F32)
            nc.sync.dma_start(out=w2sb[:], in_=w2_d[:])
            b2rsb = cst.tile([128, C], F32)
            nc.sync.dma_start(out=b2rsb[:], in_=b2r_d[:])
            betasb = cst.tile([128, 1], F32)
            nc.sync.dma_start(out=betasb[:], in_=betar_d[:])
            subsb = cst.tile([128, cols_total], F32)
            nc.sync.dma_start(out=subsb[:], in_=subv_d[:])
            degsb = cst.tile([128, NT], F32)
            nc.sync.dma_start(out=degsb[:], in_=degs_d[:])
            iotasb = cst.tile([128, KMAX], F32)
            nc.sync.dma_start(out=iotasb[:], in_=iota_d[:])
            ident128 = cst.tile([128, 128], F32)
            make_identity(nc, ident128[:])
            zer = cst.tile([1, 128], BF16)
            nc.vector.memset(zer[:], 0)
            for i in range(2):
                nc.sync.dma_start(out=tab_d[i][NW:NW + 1, :], in_=zer[:])

            # ------------- phase A: h0 = relu(x W1 + b1), node-major -------
            h0nm = st.tile([128, NT, H], F32)
            with tc.tile_pool(name="psA", bufs=1, space="PSUM") as psA, \
                 tc.tile_pool(name="psTa", bufs=4, space="PSUM") as psTa, \
                 tc.tile_pool(name="wkA", bufs=2) as wkA:
                CH = 2048
                for coff in range(0, LP, CH):
                    csz = min(CH, LP - coff)
                    ps = psA.tile([H, CH], F32, tag="psa")
                    for kt in range(12):
                        xtile = wkA.tile([128, CH], F32, tag="xt")
                        nc.sync.dma_start(
                            out=xtile[:, :csz],
                            in_=xt_d[kt * 128:(kt + 1) * 128, coff:coff + csz])
                        for m in range(0, csz, 512):
                            mw = min(512, csz - m)
                            nc.tensor.matmul(ps[:, m:m + mw],
                                             lhsT=w1sb[:, kt, :],
                                             rhs=xtile[:, m:m + mw],
                                             start=(kt == 0), stop=(kt == 11))
                    hfm = wkA.tile([H, CH], F32, tag="hfm")
                    nc.scalar.activation(hfm[:, :csz], ps[:, :csz], AF.Relu,
                                         bias=b1sb[:])
                    for i in range(csz // 128):
                        tg = (coff + i * 128) // 128
                        pt = psTa.tile([128, H], F32, tag="pst")
                        nc.tensor.transpose(
                            out=pt[:], in_=hfm[:, i * 128:(i + 1) * 128],
                            identity=ident128[:H, :H])
                        nc.vector.tensor_copy(out=h0nm[:, tg, :], in_=pt[:])

            hnbf = st.tile([128, NT, H], BF16)
            h1nm = st.tile([128, NT, H], F32)
            sgrp = st.tile([128, NT], F32)

            def normalize_and_share(hsrc, phase):
                hh = wk.tile([128, NT * H], F32, tag="hh")
                nc.scalar.activation(
                    hh[:], hsrc[:].rearrange("p t h -> p (t h)"), AF.Square)
                ss = wk.tile([128, NT], F32, tag="ss")
                nc.vector.tensor_reduce(
                    ss[:], hh[:].rearrange("p (t h) -> p t h", h=H),
                    axis=AX.X, op=OP.add)
                nc.vector.tensor_scalar_add(ss[:], ss[:], EPS)
                sq = wk.tile([128, NT], F32, tag="ss2")
                nc.scalar.activation(sq[:], ss[:], AF.Sqrt)
                rr = wk.tile([128, NT], F32, tag="rr")
                nc.vector.reciprocal(rr[:], sq[:])
                hnf = wk.tile([128, NT, H], F32, tag="hnf")
                nc.vector.tensor_tensor(
                    out=hnf[:], in0=hsrc[:],
                    in1=_bc(rr[:].unsqueeze(2), [128, NT, H]),
                    op=OP.mult)
                nc.vector.tensor_copy(out=hnbf[:], in_=hnf[:])
                rec = wk.tile([128, NT, 32], BF16, tag="rec")
                nc.vector.tensor_copy(out=rec[:, :, 0:H], in_=hnf[:])
                nc.vector.tensor_copy(out=rec[:, :, H:2 * H], in_=hsrc[:])
                nc.sync.dma_start(out=rec_d[phase][:], in_=rec[:])
                nc.gpsimd.collective_compute(
                    "AllGather", OP.bypass,
                    replica_groups=[list(range(NC_CORES))],
                    ins=[rec_d[phase][:]],
                    outs=[tab_d[phase][0:NW, :]],
                )

            def prop(hio, phase, use_beta):
                for gi, (ts, ncalls, cap, colb) in enumerate(groups):
                    idxsb = wk.tile([128, NCALLS_MAX * COL_W], I16, tag="idx")
                    cb = callbase[gi]
                    nc.sync.dma_start(
                        out=idxsb[:, :ncalls * COL_W],
                        in_=idxs_d[:, cb * COL_W:(cb + ncalls) * COL_W])
                    G4 = gp.tile([128, CAPMAX, 128], BF16, tag="g4")
                    for cc in range(ncalls):
                        nc.gpsimd.dma_gather(
                            out_ap=G4[:, cc * COLS_CALL:(cc + 1) * COLS_CALL, :],
                            in_ap=tab_d[phase][:],
                            idxs_ap=idxsb[:, cc * COL_W:(cc + 1) * COL_W],
                            num_idxs=NIDX_CALL,
                            num_idxs_reg=NIDX_CALL,
                            elem_size=128,
                            queue_num=cc % 4,
                        )
                    off = 0
                    for t in ts:
                        K = int(Kprof[t])
                        Gt = G4[:, off:off + K, :]
                        Gs = wk.tile([128, KMAX, 32], BF16, tag="gs")
                        nc.vector.tensor_copy(out=Gs[:, :K, :],
                                              in_=Gt[:, :, 0:32])
                        for j in (1, 2, 3):
                            mj = wk.tile([128, KMAX], mybir.dt.uint8, tag="mj")
                            nc.vector.tensor_scalar(
                                out=mj[:, :K],
                                in0=subsb[:, colbase[t]:colbase[t] + K],
                                scalar1=float(j), scalar2=None,
                                op0=OP.is_equal)
                            nc.vector.copy_predicated(
                                out=Gs[:, :K, :],
                                mask=_bc(mj[:, :K].unsqueeze(2),
                                         [128, K, 32]),
                                data=Gt[:, :, 32 * j:32 * j + 32])
                        prod = wk.tile([128, KMAX, H], BF16, tag="prod")
                        nc.vector.tensor_tensor(
                            out=prod[:, :K, :], in0=Gs[:, :K, 0:H],
                            in1=_bc(hnbf[:, t:t + 1, :], [128, K, H]),
                            op=OP.mult)
                        cosr = wk.tile([128, KMAX], F32, tag="cosr")
                        nc.vector.tensor_reduce(cosr[:, :K], prod[:, :K, :],
                                                axis=AX.X, op=OP.add)
                        if use_beta:
                            nc.vector.tensor_scalar_mul(cosr[:, :K],
                                                        cosr[:, :K],
                                                        betasb[:])
                        ee = wk.tile([128, KMAX], F32, tag="ee")
                        nc.scalar.activation(ee[:, :K], cosr[:, :K], AF.Exp)
                        msk = wk.tile([128, KMAX], F32, tag="msk")
                        nc.vector.tensor_tensor(
                            out=msk[:, :K],
                            in0=_bc(degsb[:, t:t + 1], [128, K]),
                            in1=iotasb[:, :K], op=OP.is_gt)
                        em = wk.tile([128, KMAX], F32, tag="em")
                        nc.vector.tensor_tensor(out=em[:, :K], in0=ee[:, :K],
                                                in1=msk[:, :K], op=OP.mult)
                        nc.vector.tensor_reduce(sgrp[:, t:t + 1], em[:, :K],
                                                axis=AX.X, op=OP.add)
                        wei = wk.tile([128, KMAX, H], BF16, tag="wei")
                        nc.vector.tensor_tensor(
                            out=wei[:, :K, :], in0=Gs[:, :K, H:2 * H],
                            in1=_bc(em[:, :K].unsqueeze(2),
                                    [128, K, H]),
                            op=OP.mult)
                        nc.vector.tensor_reduce(
                            h1nm[:, t, :],
                            wei[:, :K, :].rearrange("p k h -> p h k"),
                            axis=AX.X, op=OP.add)
                        off += K
                nc.vector.tensor_scalar_max(sgrp[:], sgrp[:], 1e-30)
                rs = wk.tile([128, NT], F32, tag="rs")
                nc.vector.reciprocal(rs[:], sgrp[:])
                nc.vector.tensor_tensor(
                    out=hio[:], in0=h1nm[:],
                    in1=_bc(rs[:].unsqueeze(2), [128, NT, H]),
                    op=OP.mult)

            normalize_and_share(h0nm, 0)
            prop(h0nm, 0, use_beta=False)
            normalize_and_share(h0nm, 1)
            prop(h0nm, 1, use_beta=True)

            # ------------- classifier + log_softmax -----------------------
            logits = st.tile([128, NT, C], F32)
            with tc.tile_pool(name="psC", bufs=2, space="PSUM") as psC:
                for t in range(NT):
                    ptx = psC.tile([H, 128], F32, tag="ptx")
                    nc.tensor.transpose(out=ptx[:], in_=h0nm[:, t, :],
                                        identity=ident128[:])
                    h2T = wk.tile([H, 128], F32, tag="h2t")
                    nc.vector.tensor_copy(out=h2T[:], in_=ptx[:])
                    pl = psC.tile([128, C], F32, tag="pl")
                    nc.tensor.matmul(pl[:], lhsT=h2T[:], rhs=w2sb[:],
                                     start=True, stop=True)
                    nc.vector.tensor_tensor(out=logits[:, t, :], in0=pl[:],
                                            in1=b2rsb[:], op=OP.add)
            m7 = wk.tile([128, NT], F32, tag="m7")
            nc.vector.tensor_reduce(m7[:], logits[:], axis=AX.X, op=OP.max)
            zm = wk.tile([128, NT, C], F32, tag="zm")
            nc.vector.tensor_tensor(
                out=zm[:], in0=logits[:],
                in1=_bc(m7[:].unsqueeze(2), [128, NT, C]),
                op=OP.subtract)
            ez = wk.tile([128, NT, C], F32, tag="ez")
            nc.scalar.activation(ez[:].rearrange("p t c -> p (t c)"),
                                 zm[:].rearrange("p t c -> p (t c)"), AF.Exp)
            s7 = wk.tile([128, NT], F32, tag="s7")
            nc.vector.tensor_reduce(s7[:], ez[:], axis=AX.X, op=OP.add)
            l7 = wk.tile([128, NT], F32, tag="l7")
            nc.scalar.activation(l7[:], s7[:], AF.Ln)
            outsb = wk.tile([128, NT, C], F32, tag="outsb")
            nc.vector.tensor_tensor(
                out=outsb[:], in0=zm[:],
                in1=_bc(l7[:].unsqueeze(2), [128, NT, C]),
                op=OP.subtract)
            nc.sync.dma_start(
                out=out_d[:].rearrange("(p t) c -> p t c", p=128),
                in_=outsb[:])

    nc.compile()
    return nc


_CACHE = {}


def kernel(x, edge_index, W1, b1, beta, W2, b2):
    x = np.asarray(x, dtype=np.float32)
    edge_index = np.asarray(edge_index)
    in_maps, meta = _host_prep(x, edge_index, np.asarray(W1), np.asarray(b1),
                               np.asarray(beta), np.asarray(W2),
                               np.asarray(b2))
    if "prog" not in _CACHE:
        _CACHE["prog"] = _build_program(meta)
    nc = _CACHE["prog"]
    res = run_bass_kernel_spmd(nc, in_maps, list(range(NC_CORES)),
                               trace=TRACE[0])
    LAST_EXEC_NS[0] = res.exec_time_ns
    out = np.empty((N, C), dtype=np.float32)
    r = np.arange(L)
    for c in range(NC_CORES):
        oc = res.results[c]["out"].reshape(128, NT, C)
        out[c * L + meta["perms"][c]] = oc[r % 128, r // 128]
    return out


# revision 7
# speedup vs baseline: 2.4944x; 1.0870x over previous
"""AGNN (2-layer propagation) Trainium2 Bass kernel, 8-core SPMD.

Sharding: destination-node ranges across 8 cores (12500 nodes/core, padded to
12544 = 98 tiles of 128), per the dst-range graph-parallel strategy. Per core:
  - h0 = relu(x_local @ W1 + b1) via PE (W1 stationary, x^T moving).
  - node records (hn||h, bf16) packed 4 nodes per 256B window -> DRAM table,
    AllGather across cores (window index fits int16 for dma_gather).
  - per-edge phase in ELL layout (dst-node-major, column-major slot stream):
    gpsimd dma_gather fetches source windows; DVE/ACT compute the
    segment softmax (shift-free: logits = beta*cos are bounded) and the
    weighted sums; per-dst reductions via tensor_reduce.
  - classifier + log_softmax on local nodes; host un-permutes and concats.
"""
import sys
import types

sys.path.insert(0, "/opt/trn_rl_repo")

import numpy as np

try:  # optional NTFF profiling hook (enabled by test.py via TRACE)
    import antenv
    if "antenv.axon_hooks" not in sys.modules:
        _hook = [None]
        _m = types.ModuleType("antenv.axon_hooks")
        _m.set_axon_ntff_profile_hook = lambda h: _hook.__setitem__(0, h)
        _m.get_axon_ntff_profile_hook = lambda: _hook[0]
        sys.modules["antenv.axon_hooks"] = _m
        antenv.axon_hooks = _m
        try:
            from trn_agent_boot.trn_boot import _ntff_profile_via_ctypes
            _m.set_axon_ntff_profile_hook(
                _ntff_profile_via_ctypes("/opt/axon/libaxon_pjrt.so"))
        except Exception:
            pass
except Exception:
    pass

import concourse.bass as bass  # noqa: F401
import concourse.mybir as mybir
import concourse.tile as tile
from concourse import bacc
from concourse import library_config
from concourse.bass_utils import run_bass_kernel_spmd
from concourse.masks import make_identity

F32 = mybir.dt.float32
BF16 = mybir.dt.bfloat16
I16 = mybir.dt.int16
AF = mybir.ActivationFunctionType
OP = mybir.AluOpType
AX = mybir.AxisListType

NC_CORES = 8
N = 100000
F_IN = 1433
H = 16
C = 7
EPS = 1e-12

L = N // NC_CORES            # 12500 local nodes per core
LP = 12544                   # padded (98 tiles of 128)
NT = LP // 128               # 98 tiles
KP = 1536                    # padded contraction dim (12 x 128)
NW_CORE = LP // 4            # 3136 windows per core
NW = NC_CORES * NW_CORE      # 25088 real windows
DUMMY_W = NW                 # zero window
NTAB = NW + 4
NIDX_CALL = 1024             # dma_gather safe per-call index count
COLS_CALL = NIDX_CALL // 128  # 8 ELL columns per call
COL_W = NIDX_CALL // 16       # 64 wrapped idx columns per call
GROUP_CAP = 88               # max summed K per gather group (cols)

TRACE = [False]
LAST_EXEC_NS = [None]


def _bc(ap, shape):
    try:
        return ap.broadcast_to(shape)
    except Exception:
        return ap.to_broadcast(shape)


def _plan(deg_sorted):
    """Shared (across cores) tile K profile and gather group/call layout."""
    Kprof = np.maximum(deg_sorted[:, ::128].max(axis=0), 1).astype(np.int64)
    groups = []          # (tile_list, ncalls, cap, col_base)
    colbase = np.zeros(NT, dtype=np.int64)
    callbase = []
    cols_total = 0
    calls_total = 0
    t = 0
    while t < NT:
        ts = [t]
        sk = int(Kprof[t])
        t += 1
        while t < NT and sk + int(Kprof[t]) <= GROUP_CAP:
            sk += int(Kprof[t])
            ts.append(t)
            t += 1
        ncalls = (sk + COLS_CALL - 1) // COLS_CALL
        cap = ncalls * COLS_CALL
        off = 0
        for tt in ts:
            colbase[tt] = cols_total + off
            off += int(Kprof[tt])
        callbase.append(calls_total)
        groups.append((ts, ncalls, cap, cols_total))
        cols_total += cap
        calls_total += ncalls
    return Kprof, groups, colbase, callbase, cols_total, calls_total


def _host_prep(x, edge_index, W1, b1, beta, W2, b2):
    deg = np.bincount(edge_index[1], minlength=N) + 1  # +self loop

    perms = []
    deg_sorted = np.zeros((NC_CORES, LP), dtype=np.int64)
    for c in range(NC_CORES):
        dl = deg[c * L:(c + 1) * L]
        order = np.argsort(-dl, kind="stable")
        perms.append(order)
        deg_sorted[c, :L] = dl[order]

    Kprof, groups, colbase, callbase, cols_total, calls_total = _plan(deg_sorted)
    KMAX = int(Kprof.max())

    src_all = np.concatenate([edge_index[0].astype(np.int64),
                              np.arange(N, dtype=np.int64)])
    dst_all = np.concatenate([edge_index[1].astype(np.int64),
                              np.arange(N, dtype=np.int64)])

    rank_of = np.empty(N, dtype=np.int64)
    for c in range(NC_CORES):
        rank_of[c * L + perms[c]] = np.arange(L)
    src_rank = rank_of[src_all]
    src_gp = (src_all // L) * LP + (src_rank % 128) * NT + (src_rank // 128)
    dstc = dst_all // L
    dst_rank = rank_of[dst_all]

    idx_streams = np.empty((NC_CORES, 128, calls_total * COL_W), dtype=np.int16)
    subv = np.zeros((NC_CORES, 128, cols_total), dtype=np.float32)

    for c in range(NC_CORES):
        sel = dstc == c
        dr = dst_rank[sel]
        gp = src_gp[sel]
        o = np.argsort(dr, kind="stable")
        dr = dr[o]; gp = gp[o]
        starts = np.searchsorted(dr, np.arange(LP))
        pos = np.arange(len(dr)) - starts[dr]
        tt = dr // 128
        pp = dr % 128
        col = colbase[tt] + pos
        iw = np.full((128, cols_total), DUMMY_W, dtype=np.int64)
        iw[pp, col] = gp >> 2
        subv[c][pp, col] = gp & 3
        for gi, (ts, ncalls, cap, colb) in enumerate(groups):
            blk = iw[:, colb:colb + cap]                   # [128, cap]
            lin = blk.T.reshape(ncalls, NIDX_CALL)         # col-major per call
            wr = lin.reshape(ncalls, COL_W, 16).transpose(0, 2, 1)
            flat = wr.transpose(1, 0, 2).reshape(16, ncalls * COL_W)
            cb = callbase[gi]
            for rep in range(8):
                idx_streams[c, rep * 16:(rep + 1) * 16,
                            cb * COL_W:(cb + ncalls) * COL_W] = flat

    deg_pt = np.zeros((NC_CORES, 128, NT), dtype=np.float32)
    for c in range(NC_CORES):
        deg_pt[c] = deg_sorted[c].reshape(NT, 128).T  # [p, t]

    iota = np.tile(np.arange(KMAX, dtype=np.float32), (128, 1))

    in_maps = []
    W1p = np.zeros((KP, H), dtype=np.float32)
    W1p[:F_IN] = W1
    for c in range(NC_CORES):
        xt = np.zeros((KP, LP), dtype=np.float32)
        xt[:F_IN, :L] = x[c * L + perms[c]].T
        in_maps.append({
            "xt": xt.astype(np.dtype("bfloat16")),
            "w1": W1p.astype(np.dtype("bfloat16")),
            "b1": b1.reshape(H, 1).astype(np.float32),
            "w2": W2.astype(np.float32),
            "b2r": np.tile(b2.reshape(1, C), (128, 1)).astype(np.float32),
            "betar": np.full((128, 1), float(beta[0]), dtype=np.float32),
            "idxs": idx_streams[c],
            "subv": subv[c],
            "degs": deg_pt[c],
            "iota": iota,
        })
    meta = dict(groups=groups, colbase=colbase, callbase=callbase,
                cols_total=cols_total, calls_total=calls_total,
                Kprof=Kprof, KMAX=KMAX, perms=perms)
    return in_maps, meta


def _build_program(meta):
    groups = meta["groups"]
    colbase = meta["colbase"]
    callbase = meta["callbase"]
    cols_total = meta["cols_total"]
    calls_total = meta["calls_total"]
    Kprof = meta["Kprof"]
    KMAX = meta["KMAX"]
    CAPMAX = int(max(g[2] for g in groups))
    NCALLS_MAX = int(max(g[1] for g in groups))

    nc = bacc.Bacc("TRN2", target_bir_lowering=False, debug=False,
                   num_devices=NC_CORES, num_swdge_queues=4)

    xt_d = nc.dram_tensor("xt", [KP, LP], BF16, kind="ExternalInput")
    w1_d = nc.dram_tensor("w1", [KP, H], BF16, kind="ExternalInput")
    b1_d = nc.dram_tensor("b1", [H, 1], F32, kind="ExternalInput")
    w2_d = nc.dram_tensor("w2", [H, C], F32, kind="ExternalInput")
    b2r_d = nc.dram_tensor("b2r", [128, C], F32, kind="ExternalInput")
    betar_d = nc.dram_tensor("betar", [128, 1], F32, kind="ExternalInput")
    idxs_d = nc.dram_tensor("idxs", [128, calls_total * COL_W], I16,
                            kind="ExternalInput")
    subv_d = nc.dram_tensor("subv", [128, cols_total], F32,
                            kind="ExternalInput")
    degs_d = nc.dram_tensor("degs", [128, NT], F32, kind="ExternalInput")
    iota_d = nc.dram_tensor("iota", [128, KMAX], F32, kind="ExternalInput")
    out_d = nc.dram_tensor("out", [LP, C], F32, kind="ExternalOutput")

    rec_d = [nc.dram_tensor(f"rec{i}", [128, NT, 32], BF16) for i in range(2)]
    tab_d = [nc.dram_tensor(f"tab{i}", [NTAB, 128], BF16, addr_space="Shared")
             for i in range(2)]

    with tile.TileContext(nc) as tc:
        with tc.tile_pool(name="const", bufs=1) as cst, \
             tc.tile_pool(name="state", bufs=1) as st, \
             tc.tile_pool(name="work", bufs=2) as wk, \
             tc.tile_pool(name="gath", bufs=2) as gp:

            nc.gpsimd.load_library(library_config.mlp)

            w1sb = cst.tile([128, 12, H], BF16)
            for kt in range(12):
                nc.sync.dma_start(out=w1sb[:, kt, :],
                                  in_=w1_d[kt * 128:(kt + 1) * 128, :])
            b1sb = cst.tile([H, 1], F32)
            nc.sync.dma_start(out=b1sb[:], in_=b1_d[:])
            w2sb = cst.tile([H, C], F32)
            nc.sync.dma_start(out=w2sb[:], in_=w2_d[:])
            b2rsb = cst.tile([128, C], F32)
            nc.sync.dma_start(out=b2rsb[:], in_=b2r_d[:])
            betasb = cst.tile([128, 1], F32)
            nc.sync.dma_start(out=betasb[:], in_=betar_d[:])
            subsb = cst.tile([128, cols_total], F32)
            nc.sync.dma_start(out=subsb[:], in_=subv_d[:])
            degsb = cst.tile([128, NT], F32)
            nc.sync.dma_start(out=degsb[:], in_=degs_d[:])
            iotasb = cst.tile([128, KMAX], F32)
            nc.sync.dma_start(out=iotasb[:], in_=iota_d[:])
            ident128 = cst.tile([128, 128], F32)
            make_identity(nc, ident128[:])
            zer = cst.tile([1, 128], BF16)
            nc.vector.memset(zer[:], 0)
            for i in range(2):
                nc.sync.dma_start(out=tab_d[i][NW:NW + 1, :], in_=zer[:])

            # ------------- phase A: h0 = relu(x W1 + b1), node-major -------
            h0nm = st.tile([128, NT, H], F32)
            with tc.tile_pool(name="psA", bufs=1, space="PSUM") as psA, \
                 tc.tile_pool(name="psTa", bufs=4, space="PSUM") as psTa, \
                 tc.tile_pool(name="wkA", bufs=2) as wkA:
                CH = 2048
                for coff in range(0, LP, CH):
                    csz = min(CH, LP - coff)
                    ps = psA.tile([H, CH], F32, tag="psa")
                    for kt in range(12):
                        xtile = wkA.tile([128, CH], BF16, tag="xt")
                        nc.sync.dma_start(
                            out=xtile[:, :csz],
                            in_=xt_d[kt * 128:(kt + 1) * 128, coff:coff + csz])
                        for m in range(0, csz, 512):
                            mw = min(512, csz - m)
                            nc.tensor.matmul(ps[:, m:m + mw],
                                             lhsT=w1sb[:, kt, :],
                                             rhs=xtile[:, m:m + mw],
                                             start=(kt == 0), stop=(kt == 11))
                    hfm = wkA.tile([H, CH], F32, tag="hfm")
                    nc.scalar.activation(hfm[:, :csz], ps[:, :csz], AF.Relu,
                                         bias=b1sb[:])
                    for i in range(csz // 128):
                        tg = (coff + i * 128) // 128
                        pt = psTa.tile([128, H], F32, tag="pst")
                        nc.tensor.transpose(
                            out=pt[:], in_=hfm[:, i * 128:(i + 1) * 128],
                            identity=ident128[:H, :H])
                        nc.vector.tensor_copy(out=h0nm[:, tg, :], in_=pt[:])

            hnbf = st.tile([128, NT, H], BF16)
            h1nm = st.tile([128, NT, H], F32)
            sgrp = st.tile([128, NT], F32)

            def normalize_and_share(hsrc, phase):
                hh = wk.tile([128, NT * H], F32, tag="hh")
                nc.scalar.activation(
                    hh[:], hsrc[:].rearrange("p t h -> p (t h)"), AF.Square)
                ss = wk.tile([128, NT], F32, tag="ss")
                nc.vector.tensor_reduce(
                    ss[:], hh[:].rearrange("p (t h) -> p t h", h=H),
                    axis=AX.X, op=OP.add)
                nc.vector.tensor_scalar_add(ss[:], ss[:], EPS)
                sq = wk.tile([128, NT], F32, tag="ss2")
                nc.scalar.activation(sq[:], ss[:], AF.Sqrt)
                rr = wk.tile([128, NT], F32, tag="rr")
                nc.vector.reciprocal(rr[:], sq[:])
                hnf = wk.tile([128, NT, H], F32, tag="hnf")
                nc.vector.tensor_tensor(
                    out=hnf[:], in0=hsrc[:],
                    in1=_bc(rr[:].unsqueeze(2), [128, NT, H]),
                    op=OP.mult)
                nc.vector.tensor_copy(out=hnbf[:], in_=hnf[:])
                rec = wk.tile([128, NT, 32], BF16, tag="rec")
                nc.vector.tensor_copy(out=rec[:, :, 0:H], in_=hnf[:])
                nc.vector.tensor_copy(out=rec[:, :, H:H + 1],
                                      in_=sq[:].unsqueeze(2))
                nc.sync.dma_start(out=rec_d[phase][:], in_=rec[:])
                nc.gpsimd.collective_compute(
                    "AllGather", OP.bypass,
                    replica_groups=[list(range(NC_CORES))],
                    ins=[rec_d[phase][:]],
                    outs=[tab_d[phase][0:NW, :]],
                )

            def prop(hio, phase, use_beta):
                for gi, (ts, ncalls, cap, colb) in enumerate(groups):
                    idxsb = wk.tile([128, NCALLS_MAX * COL_W], I16, tag="idx")
                    cb = callbase[gi]
                    nc.sync.dma_start(
                        out=idxsb[:, :ncalls * COL_W],
                        in_=idxs_d[:, cb * COL_W:(cb + ncalls) * COL_W])
                    G4 = gp.tile([128, CAPMAX, 128], BF16, tag="g4")
                    for cc in range(ncalls):
                        nc.gpsimd.dma_gather(
                            out_ap=G4[:, cc * COLS_CALL:(cc + 1) * COLS_CALL, :],
                            in_ap=tab_d[phase][:],
                            idxs_ap=idxsb[:, cc * COL_W:(cc + 1) * COL_W],
                            num_idxs=NIDX_CALL,
                            num_idxs_reg=NIDX_CALL,
                            elem_size=128,
                            queue_num=cc % 4,
                        )
                    off = 0
                    for t in ts:
                        K = int(Kprof[t])
                        Gt = G4[:, off:off + K, :]
                        Gs = wk.tile([128, KMAX, 18], BF16, tag="gs")
                        nc.vector.tensor_copy(out=Gs[:, :K, :],
                                              in_=Gt[:, :, 0:18])
                        for j in (1, 2, 3):
                            mj = wk.tile([128, KMAX], mybir.dt.uint8, tag="mj")
                            nc.vector.tensor_scalar(
                                out=mj[:, :K],
                                in0=subsb[:, colbase[t]:colbase[t] + K],
                                scalar1=float(j), scalar2=None,
                                op0=OP.is_equal)
                            nc.vector.copy_predicated(
                                out=Gs[:, :K, :],
                                mask=_bc(mj[:, :K].unsqueeze(2),
                                         [128, K, 18]),
                                data=Gt[:, :, 32 * j:32 * j + 18])
                        prod = wk.tile([128, KMAX, H], BF16, tag="prod")
                        nc.vector.tensor_tensor(
                            out=prod[:, :K, :], in0=Gs[:, :K, 0:H],
                            in1=_bc(hnbf[:, t:t + 1, :], [128, K, H]),
                            op=OP.mult)
                        cosr = wk.tile([128, KMAX], F32, tag="cosr")
                        nc.vector.tensor_reduce(cosr[:, :K], prod[:, :K, :],
                                                axis=AX.X, op=OP.add)
                        if use_beta:
                            nc.vector.tensor_scalar_mul(cosr[:, :K],
                                                        cosr[:, :K],
                                                        betasb[:])
                        ee = wk.tile([128, KMAX], F32, tag="ee")
                        nc.scalar.activation(ee[:, :K], cosr[:, :K], AF.Exp)
                        msk = wk.tile([128, KMAX], F32, tag="msk")
                        nc.vector.tensor_tensor(
                            out=msk[:, :K],
                            in0=_bc(degsb[:, t:t + 1], [128, K]),
                            in1=iotasb[:, :K], op=OP.is_gt)
                        em = wk.tile([128, KMAX], F32, tag="em")
                        nc.vector.tensor_tensor(out=em[:, :K], in0=ee[:, :K],
                                                in1=msk[:, :K], op=OP.mult)
                        nc.vector.tensor_reduce(sgrp[:, t:t + 1], em[:, :K],
                                                axis=AX.X, op=OP.add)
                        em2 = wk.tile([128, KMAX], F32, tag="em2")
                        nc.vector.tensor_tensor(out=em2[:, :K], in0=em[:, :K],
                                                in1=Gs[:, :K, H], op=OP.mult)
                        wei = wk.tile([128, KMAX, H], BF16, tag="wei")
                        nc.vector.tensor_tensor(
                            out=wei[:, :K, :], in0=Gs[:, :K, 0:H],
                            in1=_bc(em2[:, :K].unsqueeze(2),
                                    [128, K, H]),
                            op=OP.mult)
                        nc.vector.tensor_reduce(
                            h1nm[:, t, :],
                            wei[:, :K, :].rearrange("p k h -> p h k"),
                            axis=AX.X, op=OP.add)
                        off += K
                nc.vector.tensor_scalar_max(sgrp[:], sgrp[:], 1e-30)
                rs = wk.tile([128, NT], F32, tag="rs")
                nc.vector.reciprocal(rs[:], sgrp[:])
                nc.vector.tensor_tensor(
                    out=hio[:], in0=h1nm[:],
                    in1=_bc(rs[:].unsqueeze(2), [128, NT, H]),
                    op=OP.mult)

            normalize_and_share(h0nm, 0)
            prop(h0nm, 0, use_beta=False)
            normalize_and_share(h0nm, 1)
            prop(h0nm, 1, use_beta=True)

            # ------------- classifier + log_softmax -----------------------
            logits = st.tile([128, NT, C], F32)
            with tc.tile_pool(name="psC", bufs=2, space="PSUM") as psC:
                for t in range(NT):
                    ptx = psC.tile([H, 128], F32, tag="ptx")
                    nc.tensor.transpose(out=ptx[:], in_=h0nm[:, t, :],
                                        identity=ident128[:])
                    h2T = wk.tile([H, 128], F32, tag="h2t")
                    nc.vector.tensor_copy(out=h2T[:], in_=ptx[:])
                    pl = psC.tile([128, C], F32, tag="pl")
                    nc.tensor.matmul(pl[:], lhsT=h2T[:], rhs=w2sb[:],
                                     start=True, stop=True)
                    nc.vector.tensor_tensor(out=logits[:, t, :], in0=pl[:],
                                            in1=b2rsb[:], op=OP.add)
            m7 = wk.tile([128, NT], F32, tag="m7")
            nc.vector.tensor_reduce(m7[:], logits[:], axis=AX.X, op=OP.max)
            zm = wk.tile([128, NT, C], F32, tag="zm")
            nc.vector.tensor_tensor(
                out=zm[:], in0=logits[:],
                in1=_bc(m7[:].unsqueeze(2), [128, NT, C]),
                op=OP.subtract)
            ez = wk.tile([128, NT, C], F32, tag="ez")
            nc.scalar.activation(ez[:].rearrange("p t c -> p (t c)"),
                                 zm[:].rearrange("p t c -> p (t c)"), AF.Exp)
            s7 = wk.tile([128, NT], F32, tag="s7")
            nc.vector.tensor_reduce(s7[:], ez[:], axis=AX.X, op=OP.add)
            l7 = wk.tile([128, NT], F32, tag="l7")
            nc.scalar.activation(l7[:], s7[:], AF.Ln)
            outsb = wk.tile([128, NT, C], F32, tag="outsb")
            nc.vector.tensor_tensor(
                out=outsb[:], in0=zm[:],
                in1=_bc(l7[:].unsqueeze(2), [128, NT, C]),
                op=OP.subtract)
            nc.sync.dma_start(
                out=out_d[:].rearrange("(p t) c -> p t c", p=128),
                in_=outsb[:])

    nc.compile()
    return nc


_CACHE = {}


def kernel(x, edge_index, W1, b1, beta, W2, b2):
    x = np.asarray(x, dtype=np.float32)
    edge_index = np.asarray(edge_index)
    in_maps, meta = _host_prep(x, edge_index, np.asarray(W1), np.asarray(b1),
                               np.asarray(beta), np.asarray(W2),
                               np.asarray(b2))
    if "prog" not in _CACHE:
        _CACHE["prog"] = _build_program(meta)
    nc = _CACHE["prog"]
    res = run_bass_kernel_spmd(nc, in_maps, list(range(NC_CORES)),
                               trace=TRACE[0])
    LAST_EXEC_NS[0] = res.exec_time_ns
    out = np.empty((N, C), dtype=np.float32)
    r = np.arange(L)
    for c in range(NC_CORES):
        oc = res.results[c]["out"].reshape(128, NT, C)
        out[c * L + meta["perms"][c]] = oc[r % 128, r // 128]
    return out
